# revision 4
# baseline (speedup 1.0000x reference)
"""Trainium2 Bass kernel for nn_EWNE_67748814127633 (GNN message passing).

Reference computation (N=4096, IN=512, D=256, H=256, OUT=128):
    x1   = x @ lin1_w.T + lin1_b
    h    = x1 @ gat_W
    e    = leaky_relu(0.2*(h@a_self [:,None] + h@a_neigh [None,:]), 0.2)
    attn = softmax(where(adj>0, e, -9e15), axis=1)
    g    = tanh(elu(attn @ h))
    LSTM (h0=c0=0):  gates = g @ W_ih.T ; c1 = sig(i)*tanh(g) ; h1 = sig(o)*tanh(c1)
    out  = h1 @ lin2_w.T + lin2_b ; z = out / max(||out||, 1e-12)
    A_pred = sigmoid(z @ z.T)
Returns (A_pred, z, out).

Sharding: row-block over N across 8 NeuronCores (512 rows/core). Weights
replicated. h (with the a_neigh projection) and z.T are all-gathered.

Key device-side tricks:
  * masked softmax without max-subtraction: p = exp(leaky(v) + maskln) where
    maskln in {0, -200} (exp(-199..) == 0 in fp32); row softmax denominator is
    recovered for free by appending a ones-column to the attn@h matmul RHS.
  * one fused custom DVE op builds the whole exp() argument per [128,512]
    tile: (es_i + en_j)*0.2 -> leaky via max(v, 0.2v) -> + maskln.
  * exp(leaky(v)) == max(exp(v), exp(0.2v)) identity avoided: leaky computed
    directly since it is before the exp.
  * e/attn tiles live in transposed [j, i] layout so they feed the TensorE
    directly as the stationary operand (contraction over j), no transposes.
  * LSTM runs in transposed [gate-dim, i] layout so h1.T feeds lin2 directly.
"""

import numpy as np

NCORES = 8
N, IN_DIM, D, H, OUT = 4096, 512, 256, 256, 128
R = N // NCORES            # 512 rows per core
P = 128                    # partitions
NJ = N // P                # 32 j-chunks
NI = R // P                # 4 i-tiles per core
JGRP = 4                   # j-chunks fused per exp() instruction
MASK_NEG = -200.0          # exp(-200*0.2...) -> handled pre-exp; see op body
M_SCALE = 0.2              # the EWNE "M" constant == leaky slope

_EWNE_OP = None
_PROGRAM_CACHE = {}


def _get_custom_op():
    """Register (once per process) the fused edge-score DVE op:
        out = max(v, v*s1) + in0,   v = (in1 + s0) * s1
    in0 = maskln [P,N] (bf16, 0 or -200), in1 = es broadcast [P,N] f32,
    s0 = en per-partition [P,1], s1 = 0.2.
    """
    global _EWNE_OP
    if _EWNE_OP is not None:
        return _EWNE_OP
    from concourse import dve_ops
    from concourse.dve_spec import Spec, Src0, Src1, C0, C1, lower, maxx
    from concourse.dve_uop import DveOpSpec

    name = "EWNE_EDGE_EXPARG"
    if name in dve_ops._SUB_OPCODE_FOR_NAME:
        _EWNE_OP = next(o for o in dve_ops.OPS if o.name == name)
        return _EWNE_OP

    def _ref(in0, in1, s0, s1, imm2):
        v = (np.asarray(in1, np.float32) + s0) * s1
        return np.maximum(v, v * s1) + np.asarray(in0, np.float32)

    v = (Src1 + C0) * C1
    spec = Spec(body=maxx(v, v * C1) + Src0, reference=_ref)
    row = dve_ops._CUSTOM_DVE_ROW_BASE + len(dve_ops.OPS)
    assert row < 0x20, "custom-DVE opcode rows exhausted"
    dve_ops._SUB_OPCODE_FOR_NAME[name] = row
    shas = {}
    for ver in ("v3", "v4"):
        try:
            uops = lower(spec, ver=ver)
            shas[ver] = DveOpSpec(name=name, opcode=row, uops=uops, rd1_en=True).sha(
                ver
            )
        except Exception:
            pass
    op = dve_ops.DveOp(name, spec, subdim=False, uops_sha=shas)
    dve_ops.OPS.append(op)
    dve_ops.CUSTOM_DVE_SPECS[name] = spec
    _EWNE_OP = op
    return op


def _build_body(tc, nc, t):
    import concourse.mybir as mybir

    dt = mybir.dt
    f32 = dt.float32
    RG = [list(range(NCORES))]
    ewne_op = _get_custom_op()
    Act = mybir.ActivationFunctionType

    with (
        tc.tile_pool(name="consts", bufs=1) as consts,
        tc.tile_pool(name="sb", bufs=3) as sb,
        tc.tile_pool(name="dram", bufs=1, space="DRAM") as dram,
        tc.tile_pool(name="ps_misc", bufs=2, space="PSUM") as ps_misc,
    ):
        # ---- load replicated weights / constants into SBUF ----
        w1t_sb = consts.tile([P, IN_DIM // P, D], f32)          # [128,4,256]
        nc.sync.dma_start(w1t_sb[:], t["w1t"].ap().rearrange("(c p) d -> p c d", p=P))
        gw_sb = consts.tile([P, D // P, D], f32)                # [128,2,256]
        nc.sync.dma_start(gw_sb[:], t["gw"].ap().rearrange("(c p) d -> p c d", p=P))
        wiht_sb = consts.tile([P, D // P, 3 * H], f32)          # [128,2,768]
        nc.sync.dma_start(wiht_sb[:], t["wiht"].ap().rearrange("(c p) d -> p c d", p=P))
        wl2t_sb = consts.tile([P, D // P, OUT], f32)            # [128,2,128]
        nc.sync.dma_start(wl2t_sb[:], t["wl2t"].ap().rearrange("(c p) d -> p c d", p=P))
        b1_sb = consts.tile([P, D // P, 1], f32)
        nc.sync.dma_start(b1_sb[:], t["b1"].ap().rearrange("(c p) d -> p c d", p=P))
        asel_sb = consts.tile([P, D // P, 1], f32)
        nc.sync.dma_start(asel_sb[:], t["asel"].ap().rearrange("(c p) d -> p c d", p=P))
        anei_sb = consts.tile([P, D // P, 1], f32)
        nc.sync.dma_start(anei_sb[:], t["anei"].ap().rearrange("(c p) d -> p c d", p=P))
        b2bc_sb = consts.tile([P, OUT], f32)
        nc.sync.dma_start(b2bc_sb[:], t["b2bc"].ap())
        ident_sb = consts.tile([P, P], f32)
        nc.sync.dma_start(ident_sb[:], t["ident"].ap())
        neg1_sb = consts.tile([P, 1], f32)
        nc.gpsimd.memset(neg1_sb[:], -1.0)
        xT_sb = consts.tile([P, IN_DIM // P, R], f32)           # [128,4,512]
        nc.sync.dma_start(xT_sb[:], t["xT"].ap().rearrange("(c p) i -> p c i", p=P))

        # persistent single-shot intermediates
        x1t_sb = consts.tile([P, 2, R], f32)                    # x1.T  [d, i]
        ht_sb = consts.tile([P, 2, R], f32)                     # h.T   [d, i]
        gt_sb = consts.tile([P, 2, R], f32)                     # g.T   [d, i]
        h1t_sb = consts.tile([P, 2, R], f32)                    # h1.T  [H, i]
        zt_sb = consts.tile([P, R], f32)                        # z.T   [OUT, i]
        es_sb = consts.tile([1, R], f32)
        es_bc = consts.tile([P, R], f32)
        en_sb = consts.tile([1, R], f32)

        # DRAM collective bounce buffers
        h_ag_in = dram.tile([R, D + 1], f32)
        h_ag_out = dram.tile([N, D + 1], f32, addr_space="Shared")
        z_ag_in = dram.tile([P, R], f32)
        z_ag_out = dram.tile([NCORES * P, R], f32, addr_space="Shared")

        # ---- S1: x1.T = W1 @ x.T + b1 ----
        for dh in range(2):
            x1t_ps = ps_misc.tile([P, R], f32, tag="mm1")
            for kc in range(IN_DIM // P):
                nc.tensor.matmul(
                    x1t_ps[:],
                    w1t_sb[:, kc, dh * P : (dh + 1) * P],
                    xT_sb[:, kc, :],
                    start=(kc == 0),
                    stop=(kc == IN_DIM // P - 1),
                )
            nc.vector.tensor_scalar_add(x1t_sb[:, dh, :], x1t_ps[:], b1_sb[:, dh, :])

        # ---- S2: h.T = gat_W.T @ x1.T ----
        for dh in range(2):
            ht_ps = ps_misc.tile([P, R], f32, tag="mm1")
            for dc in range(2):
                nc.tensor.matmul(
                    ht_ps[:],
                    gw_sb[:, dc, dh * P : (dh + 1) * P],
                    x1t_sb[:, dc, :],
                    start=(dc == 0),
                    stop=(dc == 1),
                )
            nc.vector.tensor_copy(ht_sb[:, dh, :], ht_ps[:])

        # ---- S3: es/en row vectors; h back to natural layout for the gather ----
        es_ps = ps_misc.tile([1, R], f32, tag="mm1")
        for dc in range(2):
            nc.tensor.matmul(
                es_ps[:], asel_sb[:, dc, :], ht_sb[:, dc, :],
                start=(dc == 0), stop=(dc == 1),
            )
        nc.vector.tensor_copy(es_sb[:], es_ps[:])
        en_ps = ps_misc.tile([1, R], f32, tag="mm1")
        for dc in range(2):
            nc.tensor.matmul(
                en_ps[:], anei_sb[:, dc, :], ht_sb[:, dc, :],
                start=(dc == 0), stop=(dc == 1),
            )
        nc.vector.tensor_copy(en_sb[:], en_ps[:])
        nc.gpsimd.partition_broadcast(es_bc[:], es_sb[:])
        nc.sync.dma_start(h_ag_in[:, D : D + 1], en_sb[:])

        for dh in range(2):
            for it in range(NI):
                tp_ps = ps_misc.tile([P, P], f32, tag="mm1")
                nc.tensor.transpose(
                    tp_ps[:], ht_sb[:, dh, it * P : (it + 1) * P], ident_sb[:]
                )
                tp_sb = sb.tile([P, P], f32, tag="tp")
                nc.vector.tensor_copy(tp_sb[:], tp_ps[:])
                nc.sync.dma_start(
                    h_ag_in[it * P : (it + 1) * P, dh * P : (dh + 1) * P], tp_sb[:]
                )

        # ---- S4: AllGather h (+en column) ----
        nc.gpsimd.collective_compute(
            "AllGather",
            mybir.AluOpType.bypass,
            replica_groups=RG,
            ins=[h_ag_in.opt()],
            outs=[h_ag_out.opt()],
        )

        # ---- S5: attention: p.T tiles + attn@[h|1] accumulation ----
        with tc.tile_pool(name="ps_hp", bufs=NI, space="PSUM") as ps_hp:
            hp_ps = [
                ps_hp.tile([P, D + 1], f32, tag="hp", name=f"hp{it}")
                for it in range(NI)
            ]
            hfrs = {}
            for jc in range(NJ):
                g4 = jc % JGRP
                if g4 == 0:
                    egrp = sb.tile([P, JGRP, R], f32, tag="egrp", bufs=2)
                    pgrp = sb.tile([P, JGRP, R], f32, tag="pgrp", bufs=2)
                hfr = sb.tile([P, D + 2], f32, tag="hfr", bufs=8)
                hfrs[jc] = hfr
                nc.sync.dma_start(
                    hfr[:, 0:D], h_ag_out[jc * P : (jc + 1) * P, 0:D]
                )
                nc.sync.dma_start(
                    hfr[:, D + 1 : D + 2], h_ag_out[jc * P : (jc + 1) * P, D : D + 1]
                )
                nc.gpsimd.memset(hfr[:, D : D + 1], 1.0)
                mln_sb = sb.tile([P, R], dt.bfloat16, tag="mln", bufs=8)
                nc.sync.dma_start(mln_sb[:], t["mln"].ap()[jc * P : (jc + 1) * P, :])
                nc.vector._custom_dve(
                    ewne_op,
                    out=egrp[:, g4, :],
                    in0=mln_sb[:],
                    in1=es_bc[:],
                    s0=hfr[:, D + 1 : D + 2],
                    s1=M_SCALE,
                )
                if g4 == JGRP - 1:
                    nc.scalar.activation(pgrp[:], egrp[:], Act.Exp)
                    for j2 in range(jc - JGRP + 1, jc + 1):
                        for it in range(NI):
                            nc.tensor.matmul(
                                hp_ps[it][:],
                                pgrp[:, j2 % JGRP, it * P : (it + 1) * P],
                                hfrs[j2][:, 0 : D + 1],
                                start=(j2 == 0),
                                stop=(j2 == NJ - 1),
                            )

            # ---- S6: normalize + tanh(elu()) + transpose to g.T ----
            for it in range(NI):
                rr = sb.tile([P, 1], f32, tag="rr", bufs=4)
                nc.vector.reciprocal(rr[:], hp_ps[it][:, D : D + 1])
                hpn = sb.tile([P, D], f32, tag="hpn", bufs=4)
                nc.vector.tensor_scalar_mul(hpn[:], hp_ps[it][:, 0:D], rr[:])
                mneg = sb.tile([P, D], f32, tag="mneg", bufs=2)
                nc.vector.tensor_scalar_min(mneg[:], hpn[:], 0.0)
                em = sb.tile([P, D], f32, tag="em", bufs=2)
                nc.scalar.activation(em[:], mneg[:], Act.Exp)
                rl = sb.tile([P, D], f32, tag="rl", bufs=2)
                nc.vector.tensor_tensor(
                    rl[:], hpn[:], mneg[:], mybir.AluOpType.subtract
                )
                s_sb = sb.tile([P, D], f32, tag="s_sb", bufs=2)
                nc.vector.tensor_tensor(s_sb[:], rl[:], em[:], mybir.AluOpType.add)
                g_sb = sb.tile([P, D], f32, tag="g_sb", bufs=4)
                nc.scalar.activation(g_sb[:], s_sb[:], Act.Tanh, bias=neg1_sb[:])
                for dh in range(2):
                    gt_ps = ps_misc.tile([P, P], f32, tag="mm1")
                    nc.tensor.transpose(
                        gt_ps[:], g_sb[:, dh * P : (dh + 1) * P], ident_sb[:]
                    )
                    nc.vector.tensor_copy(gt_sb[:, dh, it * P : (it + 1) * P], gt_ps[:])

        # ---- S7: LSTM in transposed layout ----
        with tc.tile_pool(name="ps_g", bufs=2, space="PSUM") as ps_g:
            for hh in range(2):
                gates_ps = ps_g.tile([P, 3, R], f32, tag="gates")
                for k in range(3):          # i, g, o
                    col = k * H + hh * P
                    for dc in range(2):
                        nc.tensor.matmul(
                            gates_ps[:, k, :],
                            wiht_sb[:, dc, col : col + P],
                            gt_sb[:, dc, :],
                            start=(dc == 0),
                            stop=(dc == 1),
                        )
                sig_i = sb.tile([P, R], f32, tag="sig_i", bufs=2)
                nc.scalar.activation(sig_i[:], gates_ps[:, 0, :], Act.Sigmoid)
                tanh_g = sb.tile([P, R], f32, tag="tanh_g", bufs=2)
                nc.scalar.activation(tanh_g[:], gates_ps[:, 1, :], Act.Tanh)
                sig_o = sb.tile([P, R], f32, tag="sig_o", bufs=2)
                nc.scalar.activation(sig_o[:], gates_ps[:, 2, :], Act.Sigmoid)
                c1 = sb.tile([P, R], f32, tag="c1", bufs=2)
                nc.vector.tensor_tensor(
                    c1[:], sig_i[:], tanh_g[:], mybir.AluOpType.mult
                )
                tc1 = sb.tile([P, R], f32, tag="tc1", bufs=2)
                nc.scalar.activation(tc1[:], c1[:], Act.Tanh)
                nc.vector.tensor_tensor(
                    h1t_sb[:, hh, :], sig_o[:], tc1[:], mybir.AluOpType.mult
                )

            # ---- S8: out / z / z.T ----
            for it in range(NI):
                out_ps = ps_misc.tile([P, OUT], f32, tag="mm1")
                for dc in range(2):
                    nc.tensor.matmul(
                        out_ps[:],
                        h1t_sb[:, dc, it * P : (it + 1) * P],
                        wl2t_sb[:, dc, :],
                        start=(dc == 0),
                        stop=(dc == 1),
                    )
                out_sb = sb.tile([P, OUT], f32, tag="out_sb", bufs=4)
                nc.vector.tensor_tensor(
                    out_sb[:], out_ps[:], b2bc_sb[:], mybir.AluOpType.add
                )
                nc.sync.dma_start(t["out_o"].ap()[it * P : (it + 1) * P, :], out_sb[:])
                sq = sb.tile([P, OUT], f32, tag="sq", bufs=2)
                n2 = sb.tile([P, 1], f32, tag="n2", bufs=2)
                nc.scalar.activation(sq[:], out_sb[:], Act.Square, accum_out=n2[:])
                n2c = sb.tile([P, 1], f32, tag="n2c", bufs=2)
                nc.vector.tensor_scalar_max(n2c[:], n2[:], 1e-24)
                sn = sb.tile([P, 1], f32, tag="sn", bufs=2)
                nc.scalar.activation(sn[:], n2c[:], Act.Sqrt)
                rn = sb.tile([P, 1], f32, tag="rn", bufs=2)
                nc.vector.reciprocal(rn[:], sn[:])
                z_sb = sb.tile([P, OUT], f32, tag="z_sb", bufs=4)
                nc.vector.tensor_scalar_mul(z_sb[:], out_sb[:], rn[:])
                nc.sync.dma_start(t["z_o"].ap()[it * P : (it + 1) * P, :], z_sb[:])
                zt_ps = ps_misc.tile([P, P], f32, tag="mm1")
                nc.tensor.transpose(zt_ps[:], z_sb[:], ident_sb[:])
                nc.vector.tensor_copy(zt_sb[:, it * P : (it + 1) * P], zt_ps[:])

        nc.sync.dma_start(z_ag_in[:], zt_sb[:])
        # ---- S9: AllGather z.T ----
        nc.gpsimd.collective_compute(
            "AllGather",
            mybir.AluOpType.bypass,
            replica_groups=RG,
            ins=[z_ag_in.opt()],
            outs=[z_ag_out.opt()],
        )

        # ---- S10: A_pred = sigmoid(z @ z_full.T) ----
        with tc.tile_pool(name="ps_ap", bufs=2, space="PSUM") as ps_ap:
            for r in range(NCORES):
                zfr = sb.tile([P, R], f32, tag="zfr", bufs=3)
                nc.sync.dma_start(zfr[:], z_ag_out[r * P : (r + 1) * P, :])
                for ip in range(NI // 2):
                    ap_ps = ps_ap.tile([P, 2, R], f32, tag="ap")
                    for q in range(2):
                        it = ip * 2 + q
                        nc.tensor.matmul(
                            ap_ps[:, q, :],
                            zt_sb[:, it * P : (it + 1) * P],
                            zfr[:],
                            start=True,
                            stop=True,
                        )
                    apo = sb.tile([P, 2, R], f32, tag="apo", bufs=3)
                    nc.scalar.activation(apo[:], ap_ps[:], Act.Sigmoid)
                    for q in range(2):
                        it = ip * 2 + q
                        nc.sync.dma_start(
                            t["apred_o"].ap()[
                                it * P : (it + 1) * P, r * R : (r + 1) * R
                            ],
                            apo[:, q, :],
                        )


def build_program():
    """Build + compile the SPMD Bass program (identical on all 8 cores)."""
    key = "prog"
    if key in _PROGRAM_CACHE:
        return _PROGRAM_CACHE[key]
    import concourse.bacc as bacc
    import concourse.mybir as mybir
    import concourse.tile as tile

    dt = mybir.dt
    f32 = dt.float32
    nc = bacc.Bacc(
        "TRN2", target_bir_lowering=False, debug=False, num_devices=NCORES
    )
    t = {}
    t["xT"] = nc.dram_tensor("xT", [IN_DIM, R], f32, kind="ExternalInput")
    t["mln"] = nc.dram_tensor("mln", [N, R], dt.bfloat16, kind="ExternalInput")
    t["w1t"] = nc.dram_tensor("w1t", [IN_DIM, D], f32, kind="ExternalInput")
    t["b1"] = nc.dram_tensor("b1", [D, 1], f32, kind="ExternalInput")
    t["gw"] = nc.dram_tensor("gw", [D, D], f32, kind="ExternalInput")
    t["asel"] = nc.dram_tensor("asel", [D, 1], f32, kind="ExternalInput")
    t["anei"] = nc.dram_tensor("anei", [D, 1], f32, kind="ExternalInput")
    t["wiht"] = nc.dram_tensor("wiht", [D, 3 * H], f32, kind="ExternalInput")
    t["wl2t"] = nc.dram_tensor("wl2t", [D, OUT], f32, kind="ExternalInput")
    t["b2bc"] = nc.dram_tensor("b2bc", [P, OUT], f32, kind="ExternalInput")
    t["ident"] = nc.dram_tensor("ident", [P, P], f32, kind="ExternalInput")
    t["apred_o"] = nc.dram_tensor("apred_o", [R, N], f32, kind="ExternalOutput")
    t["z_o"] = nc.dram_tensor("z_o", [R, OUT], f32, kind="ExternalOutput")
    t["out_o"] = nc.dram_tensor("out_o", [R, OUT], f32, kind="ExternalOutput")

    with tile.TileContext(nc) as tc:
        _build_body(tc, nc, t)
    nc.compile()
    _PROGRAM_CACHE[key] = (nc, t)
    return nc, t


def make_in_maps(x, adj, lin1_w, lin1_b, gat_W, a_self, a_neigh, W_ih, W_hh,
                 lin2_w, lin2_b):
    """Host-side shard/layout prep. W_hh is unused (h0 == 0)."""
    import ml_dtypes

    f32 = np.float32
    x = np.ascontiguousarray(x, f32)
    shared = {
        "w1t": np.ascontiguousarray(np.asarray(lin1_w, f32).T),
        "b1": np.ascontiguousarray(np.asarray(lin1_b, f32).reshape(D, 1)),
        "gw": np.ascontiguousarray(np.asarray(gat_W, f32)),
        "asel": np.ascontiguousarray(np.asarray(a_self, f32).reshape(D, 1)),
        "anei": np.ascontiguousarray(np.asarray(a_neigh, f32).reshape(D, 1)),
        "wiht": np.ascontiguousarray(
            np.concatenate(
                [np.asarray(W_ih, f32)[0:H], np.asarray(W_ih, f32)[2 * H : 4 * H]], 0
            ).T
        ),
        "wl2t": np.ascontiguousarray(np.asarray(lin2_w, f32).T),
        "b2bc": np.ascontiguousarray(
            np.broadcast_to(np.asarray(lin2_b, f32).reshape(1, OUT), (P, OUT))
        ),
        "ident": np.eye(P, dtype=f32),
    }
    adj_pos = np.asarray(adj) > 0
    in_maps = []
    for c in range(NCORES):
        r0 = c * R
        m = dict(shared)
        m["xT"] = np.ascontiguousarray(x[r0 : r0 + R, :].T)
        m["mln"] = np.ascontiguousarray(
            np.where(adj_pos[r0 : r0 + R, :].T, f32(0.0), f32(MASK_NEG)).astype(
                ml_dtypes.bfloat16
            )
        )
        in_maps.append(m)
    return in_maps


def assemble_outputs(results):
    A_pred = np.concatenate([results[c]["apred_o"] for c in range(NCORES)], 0)
    z = np.concatenate([results[c]["z_o"] for c in range(NCORES)], 0)
    out = np.concatenate([results[c]["out_o"] for c in range(NCORES)], 0)
    return A_pred, z, out


def run_on_hw(in_maps, **kwargs):
    from concourse import bass_utils

    nc, _ = build_program()
    return bass_utils.run_bass_kernel_spmd(
        nc, in_maps, core_ids=list(range(NCORES)), **kwargs
    )


def kernel(**inputs):
    in_maps = make_in_maps(**{k: np.asarray(v) for k, v in inputs.items()})
    res = run_on_hw(in_maps)
    return assemble_outputs(res.results)


# revision 9
# speedup vs baseline: 1.1383x; 1.1383x over previous
"""Trainium2 Bass kernel for nn_EWNE_67748814127633 (GNN message passing).

Reference computation (N=4096, IN=512, D=256, H=256, OUT=128):
    x1   = x @ lin1_w.T + lin1_b
    h    = x1 @ gat_W
    e    = leaky_relu(0.2*(h@a_self [:,None] + h@a_neigh [None,:]), 0.2)
    attn = softmax(where(adj>0, e, -9e15), axis=1)
    g    = tanh(elu(attn @ h))
    LSTM (h0=c0=0):  gates = g @ W_ih.T ; c1 = sig(i)*tanh(g) ; h1 = sig(o)*tanh(c1)
    out  = h1 @ lin2_w.T + lin2_b ; z = out / max(||out||, 1e-12)
    A_pred = sigmoid(z @ z.T)
Returns (A_pred, z, out).

Sharding: row-block over N across 8 NeuronCores (512 rows/core). Weights
replicated. h (with the a_neigh projection) and z.T are all-gathered.

Key device-side tricks:
  * masked softmax without max-subtraction: p = exp(leaky(v) + maskln) where
    maskln in {0, -200} (exp(-199..) == 0 in fp32); row softmax denominator is
    recovered for free by appending a ones-column to the attn@h matmul RHS.
  * one fused custom DVE op builds the whole exp() argument per [128,512]
    tile: (es_i + en_j)*0.2 -> leaky via max(v, 0.2v) -> + maskln.
  * exp(leaky(v)) == max(exp(v), exp(0.2v)) identity avoided: leaky computed
    directly since it is before the exp.
  * e/attn tiles live in transposed [j, i] layout so they feed the TensorE
    directly as the stationary operand (contraction over j), no transposes.
  * LSTM runs in transposed [gate-dim, i] layout so h1.T feeds lin2 directly.
"""

import numpy as np

NCORES = 8
N, IN_DIM, D, H, OUT = 4096, 512, 256, 256, 128
R = N // NCORES            # 512 rows per core
P = 128                    # partitions
NJ = N // P                # 32 j-chunks
NI = R // P                # 4 i-tiles per core
JGRP = 4                   # j-chunks fused per exp() instruction
MASK_NEG = -200.0          # exp(-200*0.2...) -> handled pre-exp; see op body
M_SCALE = 0.2              # the EWNE "M" constant == leaky slope

_EWNE_OP = None
_PROGRAM_CACHE = {}


def _get_custom_op():
    """Register (once per process) the fused edge-score DVE op:
        out = max(v, v*s1) + in0,   v = (in1 + s0) * s1
    in0 = maskln [P,N] (bf16, 0 or -200), in1 = es broadcast [P,N] f32,
    s0 = en per-partition [P,1], s1 = 0.2.
    """
    global _EWNE_OP
    if _EWNE_OP is not None:
        return _EWNE_OP
    from concourse import dve_ops
    from concourse.dve_spec import Spec, Src0, Src1, C0, C1, lower, maxx
    from concourse.dve_uop import DveOpSpec

    name = "EWNE_EDGE_EXPARG"
    if name in dve_ops._SUB_OPCODE_FOR_NAME:
        _EWNE_OP = next(o for o in dve_ops.OPS if o.name == name)
        return _EWNE_OP

    def _ref(in0, in1, s0, s1, imm2):
        v = (np.asarray(in1, np.float32) + s0) * s1
        return np.maximum(v, v * s1) + np.asarray(in0, np.float32)

    v = (Src1 + C0) * C1
    spec = Spec(body=maxx(v, v * C1) + Src0, reference=_ref)
    row = dve_ops._CUSTOM_DVE_ROW_BASE + len(dve_ops.OPS)
    assert row < 0x20, "custom-DVE opcode rows exhausted"
    dve_ops._SUB_OPCODE_FOR_NAME[name] = row
    shas = {}
    for ver in ("v3", "v4"):
        try:
            uops = lower(spec, ver=ver)
            shas[ver] = DveOpSpec(name=name, opcode=row, uops=uops, rd1_en=True).sha(
                ver
            )
        except Exception:
            pass
    op = dve_ops.DveOp(name, spec, subdim=False, uops_sha=shas)
    dve_ops.OPS.append(op)
    dve_ops.CUSTOM_DVE_SPECS[name] = spec
    _EWNE_OP = op
    return op


def _build_body(tc, nc, t):
    import concourse.mybir as mybir

    dt = mybir.dt
    f32 = dt.float32
    RG = [list(range(NCORES))]
    ewne_op = _get_custom_op()
    Act = mybir.ActivationFunctionType

    with (
        tc.tile_pool(name="consts", bufs=1) as consts,
        tc.tile_pool(name="sb", bufs=3) as sb,
        tc.tile_pool(name="dram", bufs=1, space="DRAM") as dram,
        tc.tile_pool(name="ps_misc", bufs=2, space="PSUM") as ps_misc,
    ):
        # ---- load replicated weights / constants into SBUF ----
        w1t_sb = consts.tile([P, IN_DIM // P, D], f32)          # [128,4,256]
        nc.sync.dma_start(w1t_sb[:], t["w1t"].ap().rearrange("(c p) d -> p c d", p=P))
        gw_sb = consts.tile([P, D // P, D], f32)                # [128,2,256]
        nc.sync.dma_start(gw_sb[:], t["gw"].ap().rearrange("(c p) d -> p c d", p=P))
        wiht_sb = consts.tile([P, D // P, 3 * H], f32)          # [128,2,768]
        nc.sync.dma_start(wiht_sb[:], t["wiht"].ap().rearrange("(c p) d -> p c d", p=P))
        wl2t_sb = consts.tile([P, D // P, OUT], f32)            # [128,2,128]
        nc.sync.dma_start(wl2t_sb[:], t["wl2t"].ap().rearrange("(c p) d -> p c d", p=P))
        b1_sb = consts.tile([P, D // P, 1], f32)
        nc.sync.dma_start(b1_sb[:], t["b1"].ap().rearrange("(c p) d -> p c d", p=P))
        asel_sb = consts.tile([P, D // P, 1], f32)
        nc.sync.dma_start(asel_sb[:], t["asel"].ap().rearrange("(c p) d -> p c d", p=P))
        anei_sb = consts.tile([P, D // P, 1], f32)
        nc.sync.dma_start(anei_sb[:], t["anei"].ap().rearrange("(c p) d -> p c d", p=P))
        b2bc_sb = consts.tile([P, OUT], f32)
        nc.sync.dma_start(b2bc_sb[:], t["b2bc"].ap())
        ident_sb = consts.tile([P, P], f32)
        nc.sync.dma_start(ident_sb[:], t["ident"].ap())
        neg1_sb = consts.tile([P, 1], f32)
        nc.gpsimd.memset(neg1_sb[:], -1.0)
        xT_sb = consts.tile([P, IN_DIM // P, R], f32)           # [128,4,512]
        nc.sync.dma_start(xT_sb[:], t["xT"].ap().rearrange("(c p) i -> p c i", p=P))

        # persistent single-shot intermediates
        bf16 = dt.bfloat16
        x1t_sb = consts.tile([P, 2, R], f32)                    # x1.T  [d, i]
        ht_sb = consts.tile([P, 2, R], f32)                     # h.T   [d, i]
        gt_sb = consts.tile([P, 2, R], f32)                     # g.T   [d, i]
        h1t_sb = consts.tile([P, 2, R], f32)                    # h1.T  [H, i]
        zt_bf = consts.tile([P, R], bf16)                       # z.T   [OUT, i]
        es_sb = consts.tile([1, R], f32)
        es_bc = consts.tile([P, R], f32)
        en_sb = consts.tile([1, R], f32)
        p_all = consts.tile([P, NJ, R], bf16)                   # attn weights p.T

        # DRAM collective bounce buffers
        en_ag_in = dram.tile([R, 1], f32)
        en_ag_out = dram.tile([N, 1], f32, addr_space="Shared")
        h_ag_in_lo = dram.tile([R, P], bf16)
        h_ag_out_lo = dram.tile([N, P], bf16, addr_space="Shared")
        h_ag_in_hi = dram.tile([R, P], bf16)
        h_ag_out_hi = dram.tile([N, P], bf16, addr_space="Shared")
        z_ag_in = dram.tile([P, R], bf16)
        z_ag_out = dram.tile([NCORES * P, R], bf16, addr_space="Shared")

        # ---- S1: x1.T = W1 @ x.T + b1 ----
        for dh in range(2):
            x1t_ps = ps_misc.tile([P, R], f32, tag="mm1")
            for kc in range(IN_DIM // P):
                nc.tensor.matmul(
                    x1t_ps[:],
                    w1t_sb[:, kc, dh * P : (dh + 1) * P],
                    xT_sb[:, kc, :],
                    start=(kc == 0),
                    stop=(kc == IN_DIM // P - 1),
                )
            nc.vector.tensor_scalar_add(x1t_sb[:, dh, :], x1t_ps[:], b1_sb[:, dh, :])

        # ---- S2: h.T = gat_W.T @ x1.T ----
        for dh in range(2):
            ht_ps = ps_misc.tile([P, R], f32, tag="mm1")
            for dc in range(2):
                nc.tensor.matmul(
                    ht_ps[:],
                    gw_sb[:, dc, dh * P : (dh + 1) * P],
                    x1t_sb[:, dc, :],
                    start=(dc == 0),
                    stop=(dc == 1),
                )
            nc.vector.tensor_copy(ht_sb[:, dh, :], ht_ps[:])

        # ---- S3: es/en row vectors; h back to natural layout for the gather ----
        es_ps = ps_misc.tile([1, R], f32, tag="mm1")
        for dc in range(2):
            nc.tensor.matmul(
                es_ps[:], asel_sb[:, dc, :], ht_sb[:, dc, :],
                start=(dc == 0), stop=(dc == 1),
            )
        nc.vector.tensor_copy(es_sb[:], es_ps[:])
        en_ps = ps_misc.tile([1, R], f32, tag="mm1")
        for dc in range(2):
            nc.tensor.matmul(
                en_ps[:], anei_sb[:, dc, :], ht_sb[:, dc, :],
                start=(dc == 0), stop=(dc == 1),
            )
        nc.vector.tensor_copy(en_sb[:], en_ps[:])
        nc.gpsimd.partition_broadcast(es_bc[:], es_sb[:])

        # ---- S4a: tiny early AllGather of en so the attention-weight build
        # (which needs only es/en/mask) can overlap the big h AllGather ----
        nc.sync.dma_start(en_ag_in[:], en_sb[:])
        nc.gpsimd.collective_compute(
            "AllGather",
            mybir.AluOpType.bypass,
            replica_groups=RG,
            ins=[en_ag_in.opt()],
            outs=[en_ag_out.opt()],
        )

        # ---- S4b: h back to natural layout (bf16) + split lo/hi AllGathers ----
        for dh in range(2):
            h_ag_in_dh = h_ag_in_lo if dh == 0 else h_ag_in_hi
            for it in range(NI):
                tp_ps = ps_misc.tile([P, P], f32, tag="mm1")
                nc.tensor.transpose(
                    tp_ps[:], ht_sb[:, dh, it * P : (it + 1) * P], ident_sb[:]
                )
                tp_sb = sb.tile([P, P], bf16, tag="tp")
                nc.vector.tensor_copy(tp_sb[:], tp_ps[:])
                nc.sync.dma_start(h_ag_in_dh[it * P : (it + 1) * P, :], tp_sb[:])
            nc.gpsimd.collective_compute(
                "AllGather",
                mybir.AluOpType.bypass,
                replica_groups=RG,
                ins=[(h_ag_in_lo if dh == 0 else h_ag_in_hi).opt()],
                outs=[(h_ag_out_lo if dh == 0 else h_ag_out_hi).opt()],
            )

        # ---- S5a: attention weights p.T = exp(leaky+mask), overlaps h-AG ----
        for jc in range(NJ):
            g4 = jc % JGRP
            if g4 == 0:
                egrp = sb.tile([P, JGRP, R], f32, tag="egrp", bufs=2)
            enf = sb.tile([P, 1], f32, tag="enf", bufs=8)
            nc.sync.dma_start(enf[:], en_ag_out[jc * P : (jc + 1) * P, :])
            mln_sb = sb.tile([P, R], dt.bfloat16, tag="mln", bufs=8)
            nc.sync.dma_start(mln_sb[:], t["mln"].ap()[jc * P : (jc + 1) * P, :])
            nc.vector._custom_dve(
                ewne_op,
                out=egrp[:, g4, :],
                in0=mln_sb[:],
                in1=es_bc[:],
                s0=enf[:],
                s1=M_SCALE,
            )
            if g4 == JGRP - 1:
                nc.scalar.activation(
                    p_all[:, jc - JGRP + 1 : jc + 1, :], egrp[:], Act.Exp
                )

        # ---- S5b: hp = p.T.T @ [h_lo | 1] then [h_hi], accumulated in PSUM ----
        with tc.tile_pool(name="ps_hp", bufs=NI, space="PSUM") as ps_hp:
            hp_ps = [
                ps_hp.tile([P, 2 * P + 1], f32, tag="hp", name=f"hp{it}")
                for it in range(NI)
            ]
            for jc in range(NJ):
                hfr_a = sb.tile([P, P + 1], bf16, tag="hfr_a", bufs=6)
                nc.sync.dma_start(hfr_a[:, 0:P], h_ag_out_lo[jc * P : (jc + 1) * P, :])
                nc.gpsimd.memset(hfr_a[:, P : P + 1], 1.0)
                for it in range(NI):
                    nc.tensor.matmul(
                        hp_ps[it][:, 0 : P + 1],
                        p_all[:, jc, it * P : (it + 1) * P],
                        hfr_a[:],
                        start=(jc == 0),
                        stop=(jc == NJ - 1),
                    )
            for jc in range(NJ):
                hfr_b = sb.tile([P, P], bf16, tag="hfr_b", bufs=6)
                nc.sync.dma_start(hfr_b[:], h_ag_out_hi[jc * P : (jc + 1) * P, :])
                for it in range(NI):
                    nc.tensor.matmul(
                        hp_ps[it][:, P + 1 : 2 * P + 1],
                        p_all[:, jc, it * P : (it + 1) * P],
                        hfr_b[:],
                        start=(jc == 0),
                        stop=(jc == NJ - 1),
                    )

            # ---- S6: normalize + tanh(elu()) + transpose to g.T ----
            for it in range(NI):
                rr = sb.tile([P, 1], f32, tag="rr", bufs=4)
                nc.vector.reciprocal(rr[:], hp_ps[it][:, P : P + 1])
                hpn = sb.tile([P, D], f32, tag="hpn", bufs=4)
                nc.vector.tensor_scalar_mul(hpn[:, 0:P], hp_ps[it][:, 0:P], rr[:])
                nc.vector.tensor_scalar_mul(
                    hpn[:, P:D], hp_ps[it][:, P + 1 : 2 * P + 1], rr[:]
                )
                mneg = sb.tile([P, D], f32, tag="mneg", bufs=2)
                nc.vector.tensor_scalar_min(mneg[:], hpn[:], 0.0)
                em = sb.tile([P, D], f32, tag="em", bufs=2)
                nc.scalar.activation(em[:], mneg[:], Act.Exp)
                rl = sb.tile([P, D], f32, tag="rl", bufs=2)
                nc.vector.tensor_tensor(
                    rl[:], hpn[:], mneg[:], mybir.AluOpType.subtract
                )
                s_sb = sb.tile([P, D], f32, tag="s_sb", bufs=2)
                nc.vector.tensor_tensor(s_sb[:], rl[:], em[:], mybir.AluOpType.add)
                g_sb = sb.tile([P, D], f32, tag="g_sb", bufs=4)
                nc.scalar.activation(g_sb[:], s_sb[:], Act.Tanh, bias=neg1_sb[:])
                for dh in range(2):
                    gt_ps = ps_misc.tile([P, P], f32, tag="mm1")
                    nc.tensor.transpose(
                        gt_ps[:], g_sb[:, dh * P : (dh + 1) * P], ident_sb[:]
                    )
                    nc.vector.tensor_copy(gt_sb[:, dh, it * P : (it + 1) * P], gt_ps[:])

        # ---- S7: LSTM in transposed layout ----
        with tc.tile_pool(name="ps_g", bufs=2, space="PSUM") as ps_g:
            for hh in range(2):
                gates_ps = ps_g.tile([P, 3, R], f32, tag="gates")
                for k in range(3):          # i, g, o
                    col = k * H + hh * P
                    for dc in range(2):
                        nc.tensor.matmul(
                            gates_ps[:, k, :],
                            wiht_sb[:, dc, col : col + P],
                            gt_sb[:, dc, :],
                            start=(dc == 0),
                            stop=(dc == 1),
                        )
                sig_i = sb.tile([P, R], f32, tag="sig_i", bufs=2)
                nc.scalar.activation(sig_i[:], gates_ps[:, 0, :], Act.Sigmoid)
                tanh_g = sb.tile([P, R], f32, tag="tanh_g", bufs=2)
                nc.scalar.activation(tanh_g[:], gates_ps[:, 1, :], Act.Tanh)
                sig_o = sb.tile([P, R], f32, tag="sig_o", bufs=2)
                nc.scalar.activation(sig_o[:], gates_ps[:, 2, :], Act.Sigmoid)
                c1 = sb.tile([P, R], f32, tag="c1", bufs=2)
                nc.vector.tensor_tensor(
                    c1[:], sig_i[:], tanh_g[:], mybir.AluOpType.mult
                )
                tc1 = sb.tile([P, R], f32, tag="tc1", bufs=2)
                nc.scalar.activation(tc1[:], c1[:], Act.Tanh)
                nc.vector.tensor_tensor(
                    h1t_sb[:, hh, :], sig_o[:], tc1[:], mybir.AluOpType.mult
                )

            # ---- S8: out / z / z.T ----
            for it in range(NI):
                out_ps = ps_misc.tile([P, OUT], f32, tag="mm1")
                for dc in range(2):
                    nc.tensor.matmul(
                        out_ps[:],
                        h1t_sb[:, dc, it * P : (it + 1) * P],
                        wl2t_sb[:, dc, :],
                        start=(dc == 0),
                        stop=(dc == 1),
                    )
                out_sb = sb.tile([P, OUT], f32, tag="out_sb", bufs=4)
                nc.vector.tensor_tensor(
                    out_sb[:], out_ps[:], b2bc_sb[:], mybir.AluOpType.add
                )
                nc.sync.dma_start(t["out_o"].ap()[it * P : (it + 1) * P, :], out_sb[:])
                sq = sb.tile([P, OUT], f32, tag="sq", bufs=2)
                n2 = sb.tile([P, 1], f32, tag="n2", bufs=2)
                nc.scalar.activation(sq[:], out_sb[:], Act.Square, accum_out=n2[:])
                n2c = sb.tile([P, 1], f32, tag="n2c", bufs=2)
                nc.vector.tensor_scalar_max(n2c[:], n2[:], 1e-24)
                sn = sb.tile([P, 1], f32, tag="sn", bufs=2)
                nc.scalar.activation(sn[:], n2c[:], Act.Sqrt)
                rn = sb.tile([P, 1], f32, tag="rn", bufs=2)
                nc.vector.reciprocal(rn[:], sn[:])
                z_sb = sb.tile([P, OUT], f32, tag="z_sb", bufs=4)
                nc.vector.tensor_scalar_mul(z_sb[:], out_sb[:], rn[:])
                nc.sync.dma_start(t["z_o"].ap()[it * P : (it + 1) * P, :], z_sb[:])
                zt_ps = ps_misc.tile([P, P], f32, tag="mm1")
                nc.tensor.transpose(zt_ps[:], z_sb[:], ident_sb[:])
                nc.vector.tensor_copy(zt_bf[:, it * P : (it + 1) * P], zt_ps[:])

        nc.sync.dma_start(z_ag_in[:], zt_bf[:])
        # ---- S9: AllGather z.T ----
        nc.gpsimd.collective_compute(
            "AllGather",
            mybir.AluOpType.bypass,
            replica_groups=RG,
            ins=[z_ag_in.opt()],
            outs=[z_ag_out.opt()],
        )

        # ---- S10: A_pred = sigmoid(z @ z_full.T) ----
        with tc.tile_pool(name="ps_ap", bufs=2, space="PSUM") as ps_ap:
            for r in range(NCORES):
                zfr = sb.tile([P, R], bf16, tag="zfr", bufs=3)
                nc.sync.dma_start(zfr[:], z_ag_out[r * P : (r + 1) * P, :])
                for ip in range(NI // 2):
                    ap_ps = ps_ap.tile([P, 2, R], f32, tag="ap")
                    for q in range(2):
                        it = ip * 2 + q
                        nc.tensor.matmul(
                            ap_ps[:, q, :],
                            zt_bf[:, it * P : (it + 1) * P],
                            zfr[:],
                            start=True,
                            stop=True,
                        )
                    apo = sb.tile([P, 2, R], f32, tag="apo", bufs=3)
                    nc.scalar.activation(apo[:], ap_ps[:], Act.Sigmoid)
                    for q in range(2):
                        it = ip * 2 + q
                        nc.sync.dma_start(
                            t["apred_o"].ap()[
                                it * P : (it + 1) * P, r * R : (r + 1) * R
                            ],
                            apo[:, q, :],
                        )


def build_program():
    """Build + compile the SPMD Bass program (identical on all 8 cores)."""
    key = "prog"
    if key in _PROGRAM_CACHE:
        return _PROGRAM_CACHE[key]
    import concourse.bacc as bacc
    import concourse.mybir as mybir
    import concourse.tile as tile

    dt = mybir.dt
    f32 = dt.float32
    nc = bacc.Bacc(
        "TRN2", target_bir_lowering=False, debug=False, num_devices=NCORES
    )
    t = {}
    t["xT"] = nc.dram_tensor("xT", [IN_DIM, R], f32, kind="ExternalInput")
    t["mln"] = nc.dram_tensor("mln", [N, R], dt.bfloat16, kind="ExternalInput")
    t["w1t"] = nc.dram_tensor("w1t", [IN_DIM, D], f32, kind="ExternalInput")
    t["b1"] = nc.dram_tensor("b1", [D, 1], f32, kind="ExternalInput")
    t["gw"] = nc.dram_tensor("gw", [D, D], f32, kind="ExternalInput")
    t["asel"] = nc.dram_tensor("asel", [D, 1], f32, kind="ExternalInput")
    t["anei"] = nc.dram_tensor("anei", [D, 1], f32, kind="ExternalInput")
    t["wiht"] = nc.dram_tensor("wiht", [D, 3 * H], f32, kind="ExternalInput")
    t["wl2t"] = nc.dram_tensor("wl2t", [D, OUT], f32, kind="ExternalInput")
    t["b2bc"] = nc.dram_tensor("b2bc", [P, OUT], f32, kind="ExternalInput")
    t["ident"] = nc.dram_tensor("ident", [P, P], f32, kind="ExternalInput")
    t["apred_o"] = nc.dram_tensor("apred_o", [R, N], f32, kind="ExternalOutput")
    t["z_o"] = nc.dram_tensor("z_o", [R, OUT], f32, kind="ExternalOutput")
    t["out_o"] = nc.dram_tensor("out_o", [R, OUT], f32, kind="ExternalOutput")

    with tile.TileContext(nc) as tc:
        _build_body(tc, nc, t)
    nc.compile()
    _PROGRAM_CACHE[key] = (nc, t)
    return nc, t


def make_in_maps(x, adj, lin1_w, lin1_b, gat_W, a_self, a_neigh, W_ih, W_hh,
                 lin2_w, lin2_b):
    """Host-side shard/layout prep. W_hh is unused (h0 == 0)."""
    import ml_dtypes

    f32 = np.float32
    x = np.ascontiguousarray(x, f32)
    shared = {
        "w1t": np.ascontiguousarray(np.asarray(lin1_w, f32).T),
        "b1": np.ascontiguousarray(np.asarray(lin1_b, f32).reshape(D, 1)),
        "gw": np.ascontiguousarray(np.asarray(gat_W, f32)),
        "asel": np.ascontiguousarray(np.asarray(a_self, f32).reshape(D, 1)),
        "anei": np.ascontiguousarray(np.asarray(a_neigh, f32).reshape(D, 1)),
        "wiht": np.ascontiguousarray(
            np.concatenate(
                [np.asarray(W_ih, f32)[0:H], np.asarray(W_ih, f32)[2 * H : 4 * H]], 0
            ).T
        ),
        "wl2t": np.ascontiguousarray(np.asarray(lin2_w, f32).T),
        "b2bc": np.ascontiguousarray(
            np.broadcast_to(np.asarray(lin2_b, f32).reshape(1, OUT), (P, OUT))
        ),
        "ident": np.eye(P, dtype=f32),
    }
    adj_pos = np.asarray(adj) > 0
    in_maps = []
    for c in range(NCORES):
        r0 = c * R
        m = dict(shared)
        m["xT"] = np.ascontiguousarray(x[r0 : r0 + R, :].T)
        m["mln"] = np.ascontiguousarray(
            np.where(adj_pos[r0 : r0 + R, :].T, f32(0.0), f32(MASK_NEG)).astype(
                ml_dtypes.bfloat16
            )
        )
        in_maps.append(m)
    return in_maps


def assemble_outputs(results):
    A_pred = np.concatenate([results[c]["apred_o"] for c in range(NCORES)], 0)
    z = np.concatenate([results[c]["z_o"] for c in range(NCORES)], 0)
    out = np.concatenate([results[c]["out_o"] for c in range(NCORES)], 0)
    return A_pred, z, out


def run_on_hw(in_maps, **kwargs):
    from concourse import bass_utils

    nc, _ = build_program()
    return bass_utils.run_bass_kernel_spmd(
        nc, in_maps, core_ids=list(range(NCORES)), **kwargs
    )


def kernel(**inputs):
    in_maps = make_in_maps(**{k: np.asarray(v) for k, v in inputs.items()})
    res = run_on_hw(in_maps)
    return assemble_outputs(res.results)


# revision 14
# speedup vs baseline: 1.3920x; 1.2228x over previous
"""Trainium2 Bass kernel for nn_EWNE_67748814127633 (GNN message passing).

Reference computation (N=4096, IN=512, D=256, H=256, OUT=128):
    x1   = x @ lin1_w.T + lin1_b
    h    = x1 @ gat_W
    e    = leaky_relu(0.2*(h@a_self [:,None] + h@a_neigh [None,:]), 0.2)
    attn = softmax(where(adj>0, e, -9e15), axis=1)
    g    = tanh(elu(attn @ h))
    LSTM (h0=c0=0):  gates = g @ W_ih.T ; c1 = sig(i)*tanh(g) ; h1 = sig(o)*tanh(c1)
    out  = h1 @ lin2_w.T + lin2_b ; z = out / max(||out||, 1e-12)
    A_pred = sigmoid(z @ z.T)
Returns (A_pred, z, out).

Sharding: row-block over N across 8 NeuronCores (512 rows/core). Weights
replicated. h (bf16, split lo/hi) and z.T (bf16) are all-gathered on-chip.

Device-side structure:
  * the attention-score projections es = h@a_self and en = h@a_neigh are
    linear in x, so they are folded host-side into two matvecs
    (0.02% of total FLOPs); this lets the attention-weight build start
    immediately and fully overlap the h AllGather.
  * masked softmax without max-subtraction: p = exp(leaky(v) + maskln) with
    maskln in {0, -200} (exp underflows to 0); the row denominator comes from
    an extra ones-column matmul accumulated alongside attn.T @ h.
  * one fused custom DVE op builds the exp() argument per [128,512] tile:
    (es_i + en_j)*0.2 -> leaky via max(v, 0.2v) -> + maskln.
  * hp is computed transposed (h.T-block-stationary, p moving at N=512), so
    the whole LSTM runs in transposed layout with zero extra transposes.
  * all matmul operands are bf16 (fp32 matmuls cost 2 passes + 2 weight
    loads on TRN2); accumulation stays fp32 in PSUM.
"""

import numpy as np

NCORES = 8
N, IN_DIM, D, H, OUT = 4096, 512, 256, 256, 128
R = N // NCORES            # 512 rows per core
P = 128                    # partitions
NJ = N // P                # 32 j-chunks
NI = R // P                # 4 i-tiles per core
JGRP = 4                   # j-chunks fused per exp() instruction
MASK_NEG = -200.0          # pre-exp mask offset; exp(<-180) == 0 in fp32
M_SCALE = 0.2              # the EWNE "M" constant == leaky slope

_EWNE_OP = None
_PROGRAM_CACHE = {}


def _get_custom_op():
    """Register (once per process) the fused edge-score DVE op:
        out = max(v, v*s1) + in0,   v = (in1 + s0) * s1
    in0 = maskln [P,N] (bf16, 0 or -200), in1 = es broadcast [P,N] f32,
    s0 = en per-partition [P,1], s1 = 0.2.
    """
    global _EWNE_OP
    if _EWNE_OP is not None:
        return _EWNE_OP
    from concourse import dve_ops
    from concourse.dve_spec import Spec, Src0, Src1, C0, C1, lower, maxx
    from concourse.dve_uop import DveOpSpec

    name = "EWNE_EDGE_EXPARG"
    if name in dve_ops._SUB_OPCODE_FOR_NAME:
        _EWNE_OP = next(o for o in dve_ops.OPS if o.name == name)
        return _EWNE_OP

    def _ref(in0, in1, s0, s1, imm2):
        v = (np.asarray(in1, np.float32) + s0) * s1
        return np.maximum(v, v * s1) + np.asarray(in0, np.float32)

    v = (Src1 + C0) * C1
    spec = Spec(body=maxx(v, v * C1) + Src0, reference=_ref)
    row = dve_ops._CUSTOM_DVE_ROW_BASE + len(dve_ops.OPS)
    assert row < 0x20, "custom-DVE opcode rows exhausted"
    dve_ops._SUB_OPCODE_FOR_NAME[name] = row
    shas = {}
    for ver in ("v3", "v4"):
        try:
            uops = lower(spec, ver=ver)
            shas[ver] = DveOpSpec(name=name, opcode=row, uops=uops, rd1_en=True).sha(
                ver
            )
        except Exception:
            pass
    op = dve_ops.DveOp(name, spec, subdim=False, uops_sha=shas)
    dve_ops.OPS.append(op)
    dve_ops.CUSTOM_DVE_SPECS[name] = spec
    _EWNE_OP = op
    return op


def _build_body(tc, nc, t):
    import concourse.mybir as mybir

    dt = mybir.dt
    f32 = dt.float32
    bf16 = dt.bfloat16
    RG = [list(range(NCORES))]
    ewne_op = _get_custom_op()
    Act = mybir.ActivationFunctionType

    with (
        tc.tile_pool(name="consts", bufs=1) as consts,
        tc.tile_pool(name="sb", bufs=3) as sb,
        tc.tile_pool(name="dram", bufs=1, space="DRAM") as dram,
        tc.tile_pool(name="ps_misc", bufs=2, space="PSUM") as ps_misc,
    ):
        # ---- replicated weights / constants ----
        w1t_sb = consts.tile([P, IN_DIM // P, D], bf16)         # [128,4,256]
        nc.sync.dma_start(w1t_sb[:], t["w1t"].ap().rearrange("(c p) d -> p c d", p=P))
        gw_sb = consts.tile([P, D // P, D], bf16)               # [128,2,256]
        nc.sync.dma_start(gw_sb[:], t["gw"].ap().rearrange("(c p) d -> p c d", p=P))
        wiht_sb = consts.tile([P, D // P, 3 * H], bf16)         # [128,2,768]
        nc.sync.dma_start(wiht_sb[:], t["wiht"].ap().rearrange("(c p) d -> p c d", p=P))
        wl2t_sb = consts.tile([P, D // P, OUT], bf16)           # [128,2,128]
        nc.sync.dma_start(wl2t_sb[:], t["wl2t"].ap().rearrange("(c p) d -> p c d", p=P))
        b1_sb = consts.tile([P, D // P, 1], f32)
        nc.sync.dma_start(b1_sb[:], t["b1"].ap().rearrange("(c p) d -> p c d", p=P))
        b2bc_sb = consts.tile([P, OUT], f32)
        nc.sync.dma_start(b2bc_sb[:], t["b2bc"].ap())
        ident_sb = consts.tile([P, P], f32)
        nc.sync.dma_start(ident_sb[:], t["ident"].ap())
        neg1_sb = consts.tile([P, 1], f32)
        nc.gpsimd.memset(neg1_sb[:], -1.0)
        ones_bf = consts.tile([P, 1], bf16)
        nc.gpsimd.memset(ones_bf[:], 1.0)
        xT_sb = consts.tile([P, IN_DIM // P, R], bf16)          # [128,4,512]
        nc.sync.dma_start(xT_sb[:], t["xT"].ap().rearrange("(c p) i -> p c i", p=P))
        es_sb = consts.tile([1, R], f32)
        nc.sync.dma_start(es_sb[:], t["es"].ap())
        es_bc = consts.tile([P, R], f32)
        nc.gpsimd.partition_broadcast(es_bc[:], es_sb[:])

        # persistent single-shot intermediates
        x1t_sb = consts.tile([P, 2, R], bf16)                   # x1.T  [d, i]
        ht_sb = consts.tile([P, 2, R], f32)                     # h.T   [d, i]
        gt_sb = consts.tile([P, 2, R], bf16)                    # g.T   [d, i]
        h1t_sb = consts.tile([P, 2, R], bf16)                   # h1.T  [H, i]
        zt_bf = consts.tile([P, R], bf16)                       # z.T   [OUT, i]
        p_all = consts.tile([P, NJ, R], bf16)                   # attn weights p.T

        # DRAM collective bounce buffers
        h_ag_in_lo = dram.tile([R, P], bf16)
        h_ag_out_lo = dram.tile([N, P], bf16, addr_space="Shared")
        h_ag_in_hi = dram.tile([R, P], bf16)
        h_ag_out_hi = dram.tile([N, P], bf16, addr_space="Shared")
        z_ag_in = dram.tile([P, R], bf16)
        z_ag_out = dram.tile([NCORES * P, R], bf16, addr_space="Shared")

        # ---- S1: x1.T = W1 @ x.T + b1 ----
        for dh in range(2):
            x1t_ps = ps_misc.tile([P, R], f32, tag="mm1")
            for kc in range(IN_DIM // P):
                nc.tensor.matmul(
                    x1t_ps[:],
                    w1t_sb[:, kc, dh * P : (dh + 1) * P],
                    xT_sb[:, kc, :],
                    start=(kc == 0),
                    stop=(kc == IN_DIM // P - 1),
                )
            nc.vector.tensor_scalar_add(x1t_sb[:, dh, :], x1t_ps[:], b1_sb[:, dh, :])

        # ---- S2: h.T = gat_W.T @ x1.T ----
        for dh in range(2):
            ht_ps = ps_misc.tile([P, R], f32, tag="mm1")
            for dc in range(2):
                nc.tensor.matmul(
                    ht_ps[:],
                    gw_sb[:, dc, dh * P : (dh + 1) * P],
                    x1t_sb[:, dc, :],
                    start=(dc == 0),
                    stop=(dc == 1),
                )
            nc.vector.tensor_copy(ht_sb[:, dh, :], ht_ps[:])

        # ---- S3: h to natural layout (bf16) + split lo/hi AllGathers ----
        for dh in range(2):
            h_ag_in_dh = h_ag_in_lo if dh == 0 else h_ag_in_hi
            h_ag_out_dh = h_ag_out_lo if dh == 0 else h_ag_out_hi
            for it in range(NI):
                tp_ps = ps_misc.tile([P, P], f32, tag="mm1")
                nc.tensor.transpose(
                    tp_ps[:], ht_sb[:, dh, it * P : (it + 1) * P], ident_sb[:]
                )
                tp_sb = sb.tile([P, P], bf16, tag="tp")
                nc.vector.tensor_copy(tp_sb[:], tp_ps[:])
                nc.sync.dma_start(h_ag_in_dh[it * P : (it + 1) * P, :], tp_sb[:])
            nc.gpsimd.collective_compute(
                "AllGather",
                mybir.AluOpType.bypass,
                replica_groups=RG,
                ins=[h_ag_in_dh.opt()],
                outs=[h_ag_out_dh.opt()],
            )

        # ---- S4: attention weights p.T = exp(leaky+mask); overlaps h-AG ----
        for jc in range(NJ):
            g4 = jc % JGRP
            if g4 == 0:
                egrp = sb.tile([P, JGRP, R], f32, tag="egrp", bufs=2)
            enf = sb.tile([P, 1], f32, tag="enf", bufs=8)
            nc.sync.dma_start(enf[:], t["en"].ap()[jc * P : (jc + 1) * P, :])
            mln_sb = sb.tile([P, R], bf16, tag="mln", bufs=8)
            nc.sync.dma_start(mln_sb[:], t["mln"].ap()[jc * P : (jc + 1) * P, :])
            nc.vector._custom_dve(
                ewne_op,
                out=egrp[:, g4, :],
                in0=mln_sb[:],
                in1=es_bc[:],
                s0=enf[:],
                s1=M_SCALE,
            )
            if g4 == JGRP - 1:
                nc.scalar.activation(
                    p_all[:, jc - JGRP + 1 : jc + 1, :], egrp[:], Act.Exp
                )

        # ---- S5: hp.T = h.T-block @ p.T + rowsum, accumulated in PSUM ----
        with tc.tile_pool(name="ps_hp", bufs=1, space="PSUM") as ps_hp:
            hpl_ps = ps_hp.tile([P, R], f32, tag="hpl")
            hph_ps = ps_hp.tile([P, R], f32, tag="hph")
            rs_ps = ps_hp.tile([1, R], f32, tag="rs")
            for jc in range(NJ):
                hfa = sb.tile([P, P], bf16, tag="hfa", bufs=6)
                nc.sync.dma_start(hfa[:], h_ag_out_lo[jc * P : (jc + 1) * P, :])
                nc.tensor.matmul(
                    hpl_ps[:],
                    hfa[:],
                    p_all[:, jc, :],
                    start=(jc == 0),
                    stop=(jc == NJ - 1),
                )
                nc.tensor.matmul(
                    rs_ps[:],
                    ones_bf[:],
                    p_all[:, jc, :],
                    start=(jc == 0),
                    stop=(jc == NJ - 1),
                )
            for jc in range(NJ):
                hfb = sb.tile([P, P], bf16, tag="hfb", bufs=6)
                nc.sync.dma_start(hfb[:], h_ag_out_hi[jc * P : (jc + 1) * P, :])
                nc.tensor.matmul(
                    hph_ps[:],
                    hfb[:],
                    p_all[:, jc, :],
                    start=(jc == 0),
                    stop=(jc == NJ - 1),
                )

            # ---- S6: normalize + tanh(elu()) in transposed layout ----
            rr_row = sb.tile([1, R], f32, tag="rr_row")
            nc.vector.reciprocal(rr_row[:], rs_ps[:])
            rr_bc = sb.tile([P, R], f32, tag="rr_bc")
            nc.gpsimd.partition_broadcast(rr_bc[:], rr_row[:])
            hpnt = sb.tile([P, 2, R], f32, tag="hpnt")
            nc.vector.tensor_tensor(
                hpnt[:, 0, :], hpl_ps[:], rr_bc[:], mybir.AluOpType.mult
            )
            nc.vector.tensor_tensor(
                hpnt[:, 1, :], hph_ps[:], rr_bc[:], mybir.AluOpType.mult
            )
            mneg = sb.tile([P, 2, R], f32, tag="mneg")
            nc.vector.tensor_scalar_min(mneg[:], hpnt[:], 0.0)
            em = sb.tile([P, 2, R], f32, tag="em")
            nc.scalar.activation(em[:], mneg[:], Act.Exp)
            rl = sb.tile([P, 2, R], f32, tag="rl")
            nc.vector.tensor_tensor(rl[:], hpnt[:], mneg[:], mybir.AluOpType.subtract)
            s_sb = sb.tile([P, 2, R], f32, tag="s_sb")
            nc.vector.tensor_tensor(s_sb[:], rl[:], em[:], mybir.AluOpType.add)
            nc.scalar.activation(gt_sb[:], s_sb[:], Act.Tanh, bias=neg1_sb[:])

        # ---- S7: LSTM in transposed layout (bf16 matmuls) ----
        with tc.tile_pool(name="ps_g", bufs=2, space="PSUM") as ps_g:
            for hh in range(2):
                gates_ps = ps_g.tile([P, 3, R], f32, tag="gates")
                for k in range(3):          # i, g, o
                    col = k * H + hh * P
                    for dc in range(2):
                        nc.tensor.matmul(
                            gates_ps[:, k, :],
                            wiht_sb[:, dc, col : col + P],
                            gt_sb[:, dc, :],
                            start=(dc == 0),
                            stop=(dc == 1),
                        )
                sig_i = sb.tile([P, R], f32, tag="sig_i", bufs=2)
                nc.scalar.activation(sig_i[:], gates_ps[:, 0, :], Act.Sigmoid)
                tanh_g = sb.tile([P, R], f32, tag="tanh_g", bufs=2)
                nc.scalar.activation(tanh_g[:], gates_ps[:, 1, :], Act.Tanh)
                sig_o = sb.tile([P, R], f32, tag="sig_o", bufs=2)
                nc.scalar.activation(sig_o[:], gates_ps[:, 2, :], Act.Sigmoid)
                c1 = sb.tile([P, R], f32, tag="c1", bufs=2)
                nc.vector.tensor_tensor(
                    c1[:], sig_i[:], tanh_g[:], mybir.AluOpType.mult
                )
                tc1 = sb.tile([P, R], f32, tag="tc1", bufs=2)
                nc.scalar.activation(tc1[:], c1[:], Act.Tanh)
                nc.vector.tensor_tensor(
                    h1t_sb[:, hh, :], sig_o[:], tc1[:], mybir.AluOpType.mult
                )

            # ---- S8: out / z / z.T ----
            for it in range(NI):
                out_ps = ps_misc.tile([P, OUT], f32, tag="mm1")
                for dc in range(2):
                    nc.tensor.matmul(
                        out_ps[:],
                        h1t_sb[:, dc, it * P : (it + 1) * P],
                        wl2t_sb[:, dc, :],
                        start=(dc == 0),
                        stop=(dc == 1),
                    )
                out_sb = sb.tile([P, OUT], f32, tag="out_sb", bufs=4)
                nc.vector.tensor_tensor(
                    out_sb[:], out_ps[:], b2bc_sb[:], mybir.AluOpType.add
                )
                nc.sync.dma_start(t["out_o"].ap()[it * P : (it + 1) * P, :], out_sb[:])
                sq = sb.tile([P, OUT], f32, tag="sq", bufs=2)
                n2 = sb.tile([P, 1], f32, tag="n2", bufs=2)
                nc.scalar.activation(sq[:], out_sb[:], Act.Square, accum_out=n2[:])
                n2c = sb.tile([P, 1], f32, tag="n2c", bufs=2)
                nc.vector.tensor_scalar_max(n2c[:], n2[:], 1e-24)
                sn = sb.tile([P, 1], f32, tag="sn", bufs=2)
                nc.scalar.activation(sn[:], n2c[:], Act.Sqrt)
                rn = sb.tile([P, 1], f32, tag="rn", bufs=2)
                nc.vector.reciprocal(rn[:], sn[:])
                z_sb = sb.tile([P, OUT], f32, tag="z_sb", bufs=4)
                nc.vector.tensor_scalar_mul(z_sb[:], out_sb[:], rn[:])
                nc.sync.dma_start(t["z_o"].ap()[it * P : (it + 1) * P, :], z_sb[:])
                zt_ps = ps_misc.tile([P, P], f32, tag="mm1")
                nc.tensor.transpose(zt_ps[:], z_sb[:], ident_sb[:])
                nc.vector.tensor_copy(zt_bf[:, it * P : (it + 1) * P], zt_ps[:])

        nc.sync.dma_start(z_ag_in[:], zt_bf[:])
        # ---- S9: AllGather z.T ----
        nc.gpsimd.collective_compute(
            "AllGather",
            mybir.AluOpType.bypass,
            replica_groups=RG,
            ins=[z_ag_in.opt()],
            outs=[z_ag_out.opt()],
        )

        # ---- S10: A_pred = sigmoid(z @ z_full.T) ----
        with tc.tile_pool(name="ps_ap", bufs=2, space="PSUM") as ps_ap:
            for r in range(NCORES):
                zfr = sb.tile([P, R], bf16, tag="zfr", bufs=3)
                nc.sync.dma_start(zfr[:], z_ag_out[r * P : (r + 1) * P, :])
                for ip in range(NI // 2):
                    ap_ps = ps_ap.tile([P, 2, R], f32, tag="ap")
                    for q in range(2):
                        it = ip * 2 + q
                        nc.tensor.matmul(
                            ap_ps[:, q, :],
                            zt_bf[:, it * P : (it + 1) * P],
                            zfr[:],
                            start=True,
                            stop=True,
                        )
                    apo = sb.tile([P, 2, R], f32, tag="apo", bufs=3)
                    nc.scalar.activation(apo[:], ap_ps[:], Act.Sigmoid)
                    for q in range(2):
                        it = ip * 2 + q
                        nc.sync.dma_start(
                            t["apred_o"].ap()[
                                it * P : (it + 1) * P, r * R : (r + 1) * R
                            ],
                            apo[:, q, :],
                        )


def build_program():
    """Build + compile the SPMD Bass program (identical on all 8 cores)."""
    key = "prog"
    if key in _PROGRAM_CACHE:
        return _PROGRAM_CACHE[key]
    import concourse.bacc as bacc
    import concourse.mybir as mybir
    import concourse.tile as tile

    dt = mybir.dt
    f32 = dt.float32
    bf16 = dt.bfloat16
    nc = bacc.Bacc(
        "TRN2", target_bir_lowering=False, debug=False, num_devices=NCORES
    )
    t = {}
    t["xT"] = nc.dram_tensor("xT", [IN_DIM, R], bf16, kind="ExternalInput")
    t["mln"] = nc.dram_tensor("mln", [N, R], bf16, kind="ExternalInput")
    t["es"] = nc.dram_tensor("es", [1, R], f32, kind="ExternalInput")
    t["en"] = nc.dram_tensor("en", [N, 1], f32, kind="ExternalInput")
    t["w1t"] = nc.dram_tensor("w1t", [IN_DIM, D], bf16, kind="ExternalInput")
    t["b1"] = nc.dram_tensor("b1", [D, 1], f32, kind="ExternalInput")
    t["gw"] = nc.dram_tensor("gw", [D, D], bf16, kind="ExternalInput")
    t["wiht"] = nc.dram_tensor("wiht", [D, 3 * H], bf16, kind="ExternalInput")
    t["wl2t"] = nc.dram_tensor("wl2t", [D, OUT], bf16, kind="ExternalInput")
    t["b2bc"] = nc.dram_tensor("b2bc", [P, OUT], f32, kind="ExternalInput")
    t["ident"] = nc.dram_tensor("ident", [P, P], f32, kind="ExternalInput")
    t["apred_o"] = nc.dram_tensor("apred_o", [R, N], f32, kind="ExternalOutput")
    t["z_o"] = nc.dram_tensor("z_o", [R, OUT], f32, kind="ExternalOutput")
    t["out_o"] = nc.dram_tensor("out_o", [R, OUT], f32, kind="ExternalOutput")

    with tile.TileContext(nc) as tc:
        _build_body(tc, nc, t)
    nc.compile()
    _PROGRAM_CACHE[key] = (nc, t)
    return nc, t


def make_in_maps(x, adj, lin1_w, lin1_b, gat_W, a_self, a_neigh, W_ih, W_hh,
                 lin2_w, lin2_b):
    """Host-side shard/layout prep. W_hh is unused (h0 == 0)."""
    import ml_dtypes

    f32 = np.float32
    bf = ml_dtypes.bfloat16
    x = np.ascontiguousarray(x, f32)
    lin1_w = np.asarray(lin1_w, f32)
    lin1_b = np.asarray(lin1_b, f32)
    gat_W = np.asarray(gat_W, f32)
    W_ih = np.asarray(W_ih, f32)

    # fold the attention-score projections: es/en = x @ (W1.T gat_W a) + b-term
    ga_s = gat_W @ np.asarray(a_self, f32)
    ga_n = gat_W @ np.asarray(a_neigh, f32)
    x64 = x.astype(np.float64)
    es_full = (x64 @ (lin1_w.T @ ga_s).astype(np.float64)
               + float(lin1_b @ ga_s)).astype(f32)
    en_full = (x64 @ (lin1_w.T @ ga_n).astype(np.float64)
               + float(lin1_b @ ga_n)).astype(f32)

    shared = {
        "w1t": np.ascontiguousarray(lin1_w.T.astype(bf)),
        "b1": np.ascontiguousarray(lin1_b.reshape(D, 1)),
        "gw": np.ascontiguousarray(gat_W.astype(bf)),
        "wiht": np.ascontiguousarray(
            np.concatenate([W_ih[0:H], W_ih[2 * H : 4 * H]], 0).T.astype(bf)
        ),
        "wl2t": np.ascontiguousarray(np.asarray(lin2_w, f32).T.astype(bf)),
        "b2bc": np.ascontiguousarray(
            np.broadcast_to(np.asarray(lin2_b, f32).reshape(1, OUT), (P, OUT))
        ),
        "ident": np.eye(P, dtype=f32),
        "en": np.ascontiguousarray(en_full.reshape(N, 1)),
    }
    adj_pos = np.asarray(adj) > 0
    in_maps = []
    for c in range(NCORES):
        r0 = c * R
        m = dict(shared)
        m["xT"] = np.ascontiguousarray(x[r0 : r0 + R, :].T.astype(bf))
        m["es"] = np.ascontiguousarray(es_full[r0 : r0 + R].reshape(1, R))
        m["mln"] = np.ascontiguousarray(
            np.where(adj_pos[r0 : r0 + R, :].T, f32(0.0), f32(MASK_NEG)).astype(bf)
        )
        in_maps.append(m)
    return in_maps


def assemble_outputs(results):
    A_pred = np.concatenate([results[c]["apred_o"] for c in range(NCORES)], 0)
    z = np.concatenate([results[c]["z_o"] for c in range(NCORES)], 0)
    out = np.concatenate([results[c]["out_o"] for c in range(NCORES)], 0)
    return A_pred, z, out


def run_on_hw(in_maps, **kwargs):
    from concourse import bass_utils

    nc, _ = build_program()
    return bass_utils.run_bass_kernel_spmd(
        nc, in_maps, core_ids=list(range(NCORES)), **kwargs
    )


def kernel(**inputs):
    in_maps = make_in_maps(**{k: np.asarray(v) for k, v in inputs.items()})
    res = run_on_hw(in_maps)
    return assemble_outputs(res.results)
